# revision 43
# baseline (speedup 1.0000x reference)
"""DeepseekV3 attention (B=1, S=2048, D=2048, H=16, KV=4) on 8 trn2 cores.

Sharding: query tokens modulo-8 (core c owns {t : t%8==c}, 256 each, for
causal load balance); kv_a is sharded by CONTIGUOUS token blocks (core c
computes the compressed ckv for tokens [256c, 256c+256)), rms-scaled and
roped locally, then AllGather'd (576x256 bf16 per core) so keys stay in
natural order and the causal band-mask trick survives. kv_b (k_nope / v
expansion) runs on the gathered ckv on every core; q path is fully
token-local. o_proj is token-local.

vs the previous version: kv_a replication removed (-143K PE cyc/core), all
weight/activation DMAs are contiguous tile-stacks with ~60 big triggers
instead of ~370 small ones (trigger cost ~600ns engine time each), wo is
prefetched during the q/kv_b window, softmax reciprocals stay on
the DVE reciprocal (approx custom-DVE ops don't compile on this walrus), and all partition-broadcast outer-products run
bf16 (1 cyc/row vs 4).
"""
import math
import sys
import types

import numpy as np
from ml_dtypes import bfloat16

# ---------------------------------------------------------------------------
# Container compat: this walrus build rejects instructions carrying more than
# one sync-wait command. Patch Tile to (a) split multi-wait instructions into
# single-wait NoOps on the same engine, (b) hoist the end-of-kernel drain's
# waits onto single-wait NOPs. Also register the NTFF profile hook (the
# image's antenv lacks axon_hooks) so trace=True works for profiling.
# ---------------------------------------------------------------------------
import concourse.bass as bass
import concourse.mybir as mybir
import concourse.tile as tile
from concourse.bass_utils import run_bass_kernel_spmd
from concourse.tile import ScopedClock
from bass_rust import VectorClock

N_PROCS = len(VectorClock())
_PATCHED = False


def _install_ntff_hook():
    if 'antenv.axon_hooks' in sys.modules:
        return
    m = types.ModuleType('antenv.axon_hooks')
    holder = [None]
    m.set_axon_ntff_profile_hook = lambda h: holder.__setitem__(0, h)
    m.get_axon_ntff_profile_hook = lambda: holder[0]
    sys.modules['antenv.axon_hooks'] = m
    try:
        from trn_agent_boot.trn_boot import _ntff_profile_via_ctypes
        m.set_axon_ntff_profile_hook(
            _ntff_profile_via_ctypes('/opt/axon/libaxon_pjrt.so'))
    except Exception:
        pass


def _patched_drain_and_barrier(self, tick_clock, wait_clock):
    gc = tick_clock.global_clock
    for p in range(N_PROCS):
        if gc[p] == 0:
            continue
        single = VectorClock([gc[q] if q == p else 0 for q in range(N_PROCS)])
        nop_inst = self.nc.sync.nop(nofuse=True)
        wait_clock.add_sem_waits(nop_inst.ins, ScopedClock({None: single}))
    self.nc.sync.drain()
    self.nc.all_engine_barrier()
    popped = self.nc._tile_sem_poison_stack.pop()
    assert popped is self._sem_poison
    self.nc.clear_and_free_semaphores(list(self.sems.allocated().values()))
    self.nc.all_engine_barrier()


def _make_split_lower(orig):
    def _split_multi_waits(self, ordered):
        nc = self.nc
        for bb_name, insts in ordered.items():
            out = []
            for inst in insts:
                si = inst.sync_info
                waits = list(si.on_wait) if si is not None else []
                if len(waits) > 1:
                    for w in waits[:-1]:
                        nop = mybir.InstNoOp(
                            name=f"{inst.name}-waitsplit-{nc.next_id()}",
                            engine=inst.engine,
                            sync_info=mybir.SyncInfo(on_wait=[w], on_update=[]),
                        )
                        nc.register_instruction(nop)
                        out.append(nop)
                    inst.sync_info = mybir.SyncInfo(
                        on_wait=[waits[-1]], on_update=list(si.on_update))
                out.append(inst)
            ordered[bb_name] = out
        return orig(self, ordered)
    return _split_multi_waits


def _install_patches():
    global _PATCHED
    _install_ntff_hook()
    if _PATCHED:
        return
    tile.TileContext._drain_and_barrier = _patched_drain_and_barrier
    tile.TileContext._lower_ordered_insts = _make_split_lower(
        tile.TileContext._lower_ordered_insts)
    _PATCHED = True


_install_patches()

# ---------------------------------------------------------------------------
# Problem constants (hardcoded per the spec).
# ---------------------------------------------------------------------------
S = 2048
D = 2048
H = 16
KV = 4
DN = 128          # d_nope
DR = 64           # d_rope
DQK = DN + DR     # 192
DV = 128
QR = 1536         # q rank
KVR = 512         # kv rank
EPS = 1e-6
NC_ = 8           # cores
TPC = S // NC_    # 256 tokens per core
NB = S // 128     # 16 k-subtiles
SCALE = 1.0 / math.sqrt(DQK)
NEG = -1e30

F32 = mybir.dt.float32
BF16 = mybir.dt.bfloat16
AF = mybir.ActivationFunctionType

_BUILT = None     # cached (nc,) so repeat kernel() calls skip rebuild
LAST_RESULTS = None  # BassKernelResults stash for test.py


def _ap3(dram, row_off, nrow, ngrp, grp_stride, ncol):
    """[nrow, ngrp, ncol] view of a row-major dram tensor: partition p =
    row row_off+p (stride ncol), free dims (group with stride grp_stride,
    col with stride 1)."""
    ap = dram[:]
    return bass.AP(tensor=ap.tensor, offset=ap.offset + row_off * ncol,
                   ap=[[ncol, nrow], [grp_stride, ngrp], [1, ncol]])


def _build():
    nc = bass.Bass(num_devices=NC_)

    # ---- DRAM I/O (identical declaration on all cores; data differs) ----
    hq_d = nc.dram_tensor("hq", [128, 16, TPC], BF16, kind="ExternalInput")
    hkv_d = nc.dram_tensor("hkv", [128, 16, TPC], BF16, kind="ExternalInput")
    wqa_d = nc.dram_tensor("wqa", [4, 2, 128, 4, 768], BF16,
                           kind="ExternalInput")
    wqb_d = nc.dram_tensor("wqb", [4, 2, 128, 6, 768], BF16,
                           kind="ExternalInput")
    wkva_d = nc.dram_tensor("wkva", [4, 128, 4, KVR + DR], BF16,
                            kind="ExternalInput")
    wkvb_d = nc.dram_tensor("wkvb", [128, 4, 2, 512], BF16,
                            kind="ExternalInput")
    wo_d = nc.dram_tensor("wo", [8, 128, 2, 4, 512], BF16,
                          kind="ExternalInput")
    cossin_kv_d = nc.dram_tensor("cossin_kv", [64, 2, TPC], BF16,
                                 kind="ExternalInput")
    cosq_d = nc.dram_tensor("cosq", [128, TPC], BF16, kind="ExternalInput")
    sinq_d = nc.dram_tensor("sinq", [128, TPC], BF16, kind="ExternalInput")
    bmask_d = nc.dram_tensor("bmask", [128, 4, 16], F32, kind="ExternalInput")
    out_d = nc.dram_tensor("out", [2, 4, 128, 512], F32, kind="ExternalOutput")
    cc_in = nc.dram_tensor("cc_in", [KVR + DR, TPC], BF16, kind="Internal")
    cc_out = nc.dram_tensor("cc_out", [NC_, KVR + DR, TPC], BF16,
                            kind="Internal", addr_space="Shared")

    with tile.TileContext(nc) as tc:
        with (
            tc.tile_pool(name="persist", bufs=1) as P,
            tc.tile_pool(name="ppool", bufs=3) as PP,
        ):
            ones_b = P.tile([128, 1], BF16, name="ones_b")
            nc.vector.memset(ones_b[:], 1.0)
            # [1,128] bf16 ones row: lhsT of PE outer-products used to
            # broadcast a [1,N] row across all 128 partitions (bf16 both
            # operands -> 1 cyc/row)
            ones_row = P.tile([1, 128], BF16, name="ones_row")
            nc.vector.memset(ones_row[:], 1.0)
            eps_sb = P.tile([1, 1], F32, name="eps_sb")
            nc.vector.memset(eps_sb[:], EPS)
            # band mask [k, slot, i]: 0 if k <= 8i + c else NEG (same for
            # every key block kb; applies to query cols 16kb..16kb+16)
            bmask_sb = P.tile([128, 4, 16], F32, name="bmask_sb")
            nc.scalar.dma_start(bmask_sb[:], bmask_d[:, :, :])
            cossin_kv = P.tile([64, 2, TPC], BF16, name="cossin_kv")
            nc.scalar.dma_start(cossin_kv[:], cossin_kv_d[:, :, :])
            cosq_sb = P.tile([128, TPC], BF16, name="cosq_sb")
            sinq_sb = P.tile([128, TPC], BF16, name="sinq_sb")

            # attention-lived products, grouped by kv head (4 slots = heads
            # 4*hk..4*hk+3; rope slots zero-padded in complementary halves).
            # qn_hk doubles as the attention-output tile (normalized AV is
            # written back over it once the last scores matmul has read it).
            qn_hk = [P.tile([128, 4, TPC], BF16, name=f"qn{g}")
                     for g in range(4)]
            qr_hk = [P.tile([128, 4, TPC], BF16, name=f"qr{g}")
                     for g in range(4)]
            for g in range(4):
                nc.vector.memset(qr_hk[g][:], 0.0)
            knopeT = [P.tile([128, S], BF16, name=f"knopeT{h}")
                      for h in range(KV)]
            v_sb = [P.tile([128, KV * DV], BF16, name=f"v{m}")
                    for m in range(NB)]
            # k_rot^T duplicated in both partition halves so the rope scores
            # matmul serves both heads of a pair in one 128-contraction
            krot2 = P.tile([128, S], BF16, name="krot2")

            # ================= kv_a (local 256 tokens) + Q + kv_b =============
            with (
                tc.tile_pool(name="qwin", bufs=1) as QW,
                tc.tile_pool(name="qaw", bufs=3) as QA,
                tc.tile_pool(name="qsc", bufs=2) as QS,
                tc.tile_pool(name="qbw", bufs=2) as QB,
            ):
                # early streams: hkv gates kv_a (first on sync), hq gates q_a;
                # wqa quarters ride gpsimd ahead of the collective trigger
                hkv_ctx = tc.tile_pool(name="hkvp", bufs=1)
                HKV = hkv_ctx.__enter__()
                hkv = HKV.tile([128, 16, TPC], BF16, name="hkv")
                for q in range(4):
                    nc.sync.dma_start(hkv[:, 4 * q:4 * q + 4, :],
                                      hkv_d[:, 4 * q:4 * q + 4, :])
                hq_sb = QW.tile([128, 16, TPC], BF16, name="hq")
                # wqa in 8 half-column granules (q-quarter x rank-half),
                # each consumed exactly once by the corresponding q_a half
                # pass. The first three (plus hq/cos/sin) ride the gpsimd
                # queue so sync carries only hkv and scalar only wkva in the
                # critical first microseconds; the rest are triggered lazily
                # inside q_a as ring slots free up.
                wqa_sb = [QA.tile([128, 4, 768], BF16, name="wqa", tag="wqa")
                          for i in range(8)]
                for i in range(3):
                    nc.gpsimd.dma_start(wqa_sb[i][:],
                                        wqa_d[i % 4, i // 4, :, :, :])
                nc.gpsimd.dma_start(hq_sb[:], hq_d[:, :, :])
                nc.gpsimd.dma_start(cosq_sb[:], cosq_d[:, :])
                nc.gpsimd.dma_start(sinq_sb[:], sinq_d[:, :])
                wkvb_sb = QW.tile([128, 4, 2, 512], BF16, name="wkvb")
                # kv_a scratch that survives into early q_a: the scale
                # broadcast + scaled cc_in shipping + collective launch are
                # interleaved with the first q_a k-steps so the PE never
                # stalls on the (serial) sqrt/reciprocal chain
                ckvr = [QW.tile([128, 2, TPC], BF16, name=f"ckvr{t}")
                        for t in range(2)]
                srt_k = QW.tile([1, TPC], F32, name="srt_k")
                rec_k = QW.tile([1, TPC], F32, name="rec_k")
                recb_k = QW.tile([1, TPC], BF16, name="recb_k")
                kscale = QW.tile([128, TPC], F32, name="kscale")
                kro = QW.tile([64, TPC], BF16, name="kro")

                with (
                    tc.tile_pool(name="kvaw", bufs=1) as KW,
                    tc.tile_pool(name="kvas", bufs=1) as KS,
                    tc.tile_pool(name="kvap", bufs=1, space="PSUM") as PSK,
                ):
                    wkva_sb = [KW.tile([128, 4, KVR + DR], BF16,
                                       name=f"wkva{q}") for q in range(4)]
                    for q in range(4):
                        nc.scalar.dma_start(wkva_sb[q][:], wkva_d[q, :, :, :])
                    nc.scalar.dma_start(wkvb_sb[:], wkvb_d[:, :, :, :])

                    ps = [PSK.tile([128, TPC], F32, name=f"ps_kva{m}")
                          for m in range(4)]
                    ps_r = PSK.tile([64, TPC], F32, name="ps_kvar")
                    ps_ss = PSK.tile([128, TPC], F32, name="ps_ssk")
                    # m-outer: each rank group completes early; its raw bf16
                    # evac feeds the squares so the sum-of-squares row is
                    # done moments after the last matmul
                    for m in range(4):
                        for k in range(16):
                            q, kk = divmod(k, 4)
                            nc.tensor.matmul(
                                ps[m][:],
                                wkva_sb[q][:, kk, m * 128:(m + 1) * 128],
                                hkv[:, k, :], start=(k == 0), stop=(k == 15))
                        nc.vector.tensor_copy(ckvr[m // 2][:, m % 2, :],
                                              ps[m][:])
                        sq = KS.tile([128, TPC], BF16, name="sqk", tag="sqk",
                                     bufs=2)
                        nc.vector.tensor_mul(sq[:], ckvr[m // 2][:, m % 2, :],
                                             ckvr[m // 2][:, m % 2, :])
                        nc.tensor.matmul(ps_ss[0:1, :], ones_b[:], sq[:],
                                         start=(m == 0), stop=(m == 3))
                    for k in range(16):
                        q, kk = divmod(k, 4)
                        nc.tensor.matmul(ps_r[:], wkva_sb[q][:, kk, 512:576],
                                         hkv[:, k, :], start=(k == 0),
                                         stop=(k == 15))
                    # RoPE on k_rot (raw; no rms on the rope part)
                    ck4 = KS.tile([64, TPC], BF16, name="ck4")
                    nc.vector.tensor_copy(ck4[:], ps_r[:])
                    kxr = KS.tile([64, TPC], BF16, name="kxr")
                    nc.sync.dma_start(kxr[0:32, :], ck4[32:64, :])
                    nc.sync.dma_start(kxr[32:64, :], ck4[0:32, :])
                    kt1 = KS.tile([64, TPC], BF16, name="kt1")
                    nc.vector.tensor_mul(kt1[:], ck4[:],
                                         cossin_kv[:, 0, :])
                    nc.vector.tensor_mul(kxr[:], kxr[:], cossin_kv[:, 1, :])
                    nc.vector.tensor_add(kro[:], kt1[:], kxr[:])
                    # serial scale chain (Act+DVE, runs as deps land)
                    nc.scalar.activation(srt_k[:], ps_ss[0:1, :], AF.Sqrt,
                                         bias=eps_sb[:], scale=1.0 / KVR)
                    nc.vector.reciprocal(rec_k[:], srt_k[:])
                    nc.vector.tensor_copy(recb_k[:], rec_k[:])
                hkv_ctx.__exit__(None, None, None)

                # ===================== Q window =====================
                # wqb streams in half-g granules on a 3-deep ring: the first
                # three ride sync behind the cc_in writes; the rest are
                # triggered lazily inside the g-loop (a ring slot only frees
                # once q_b consumes it, and an early trigger would stall its
                # engine ahead of work q_b itself depends on)
                wqb_sb = [QB.tile([128, 6, 768], BF16, name="wqb", tag="wqb",
                                  bufs=3) for i in range(8)]
                for i in range(3):
                    nc.sync.dma_start(wqb_sb[i][:],
                                      wqb_d[i // 2, i % 2, :, :, :])
                qaT = [QW.tile([128, TPC], BF16, name=f"qaT{m}")
                       for m in range(12)]
                with tc.tile_pool(name="qaps", bufs=1, space="PSUM") as PSQ:
                    # 12 accumulation groups don't fit 8 psum banks (one
                    # pending group per 2KB zero region), so run two half
                    # passes over 6 single-group tiles: raw bf16 evac + the
                    # squares between passes; the rms scale commutes through
                    # the q_b contraction and is applied at the q_b evacs.
                    # ps_bc is shared sequentially by the kv and q scale
                    # broadcasts (1 spare bank, groups never overlap).
                    pss = [PSQ.tile([128, TPC], F32, name=f"ps_qa{b}")
                           for b in range(6)]
                    ps_qss = PSQ.tile([128, TPC], F32, name="ps_qss")
                    ps_bc = PSQ.tile([128, TPC], F32, name="ps_bc")
                    for half in range(2):
                        for k in range(16):
                            q, kk = divmod(k, 4)
                            idx = 4 * half + q
                            if kk == 0 and idx + 3 < 8:
                                i = idx + 3
                                nc.sync.dma_start(
                                    wqa_sb[i][:], wqa_d[i % 4, i // 4, :, :, :])
                            for m6 in range(6):
                                nc.tensor.matmul(
                                    pss[m6][:],
                                    wqa_sb[idx][:, kk,
                                                m6 * 128:(m6 + 1) * 128],
                                    hq_sb[:, k, :], start=(k == 0),
                                    stop=(k == 15))
                            if half == 0 and k == 2:
                                # kv scale broadcast + scaled cc_in shipping
                                # + gather launch, hidden under q_a compute
                                nc.tensor.matmul(ps_bc[:], ones_row[:],
                                                 recb_k[:], start=True,
                                                 stop=True)
                                nc.vector.tensor_copy(kscale[:], ps_bc[:])
                                for m in range(4):
                                    nc.vector.tensor_mul(
                                        ckvr[m // 2][:, m % 2, :],
                                        ckvr[m // 2][:, m % 2, :], kscale[:])
                                for t in range(2):
                                    nc.sync.dma_start(
                                        _ap3(cc_in, 256 * t, 128, 2,
                                             128 * TPC, TPC), ckvr[t][:])
                                nc.sync.dma_start(cc_in[512:576, :], kro[:])
                                nc.gpsimd.collective_compute(
                                    "AllGather", mybir.AluOpType.bypass,
                                    replica_groups=[list(range(NC_))],
                                    ins=[cc_in[:, :]],
                                    outs=[cc_out[:, :, :]])
                        for m6 in range(6):
                            m = 6 * half + m6
                            nc.vector.tensor_copy(qaT[m][:], pss[m6][:])
                            sq = QS.tile([128, TPC], BF16, name="sqq",
                                         tag="sqq")
                            nc.vector.tensor_mul(sq[:], qaT[m][:], qaT[m][:])
                            nc.tensor.matmul(ps_qss[0:1, :], ones_b[:], sq[:],
                                             start=(m == 0), stop=(m == 11))
                    srt_q = QS.tile([1, TPC], F32, name="srt_q", bufs=1)
                    nc.scalar.activation(srt_q[:], ps_qss[0:1, :], AF.Sqrt,
                                         bias=eps_sb[:], scale=1.0 / QR)
                    rec_q = QS.tile([1, TPC], F32, name="rec_q", bufs=1)
                    nc.vector.reciprocal(rec_q[:], srt_q[:])
                    recb_q = QS.tile([1, TPC], BF16, name="recb_q", bufs=1)
                    nc.vector.tensor_copy(recb_q[:], rec_q[:])
                    nc.tensor.matmul(ps_bc[:], ones_row[:], recb_q[:],
                                     start=True, stop=True)
                    qscale = QW.tile([128, TPC], F32, name="qscale")
                    nc.vector.tensor_copy(qscale[:], ps_bc[:])

                # gathered-ckv tiles: loads ride gpsimd right behind the
                # collective so no other engine stalls waiting on it
                ckv_ctx = tc.tile_pool(name="ckvp", bufs=1)
                CKV = ckv_ctx.__enter__()
                ckvT = [CKV.tile([128, NC_, TPC], BF16, name=f"ckvT{m}")
                        for m in range(4)]
                for m in range(4):
                    nc.gpsimd.dma_start(
                        ckvT[m][:],
                        _ap3(cc_out, 128 * m, 128, NC_, 576 * TPC, TPC))
                for half in range(2):
                    nc.gpsimd.dma_start(
                        krot2[64 * half:64 * half + 64, :].rearrange(
                            "p (g j) -> p g j", g=NC_),
                        _ap3(cc_out, 512, 64, NC_, 576 * TPC, TPC))

                # q_b: per kv-head group g: 4 nope heads + 2 rope pairs
                with tc.tile_pool(name="qbps", bufs=1, space="PSUM") as PSB:
                    for g in range(4):
                        for i in (2 * g + 3, 2 * g + 4):
                            if i < 8:
                                nc.scalar.dma_start(
                                    wqb_sb[i][:], wqb_d[i // 2, i % 2, :, :, :])
                        psn = [PSB.tile([128, TPC], F32, name=f"ps_qb{u}",
                                        tag=f"ps_qb{u}") for u in range(6)]
                        for k in range(12):
                            hf, kc = divmod(k, 6)
                            for u in range(6):
                                nc.tensor.matmul(
                                    psn[u][:],
                                    wqb_sb[2 * g + hf][:, kc,
                                                       u * 128:(u + 1) * 128],
                                    qaT[k][:], start=(k == 0), stop=(k == 11))
                        for l in range(4):
                            nc.vector.tensor_mul(qn_hk[g][:, l, :],
                                                 psn[l][:], qscale[:])
                        # RoPE on q pairs (rows 0-63 head 4g+2lj, 64-127 head
                        # 4g+2lj+1): out = x*cos2 + rot(x)*sin2, rot =
                        # partition rotate by 32 within each 64-row block
                        # (sbuf DMA), rotate_half sign folded into sin2
                        # host-side. Results go into zero-padded slots.
                        for lj in range(2):
                            tmp = QS.tile([128, TPC], BF16, name="tmpr",
                                          tag="tmpr")
                            nc.vector.tensor_mul(tmp[:], psn[4 + lj][:],
                                                 qscale[:])
                            xr = QS.tile([128, TPC], BF16, name="xr", tag="xr")
                            for b0, b1 in ((0, 32), (32, 0), (64, 96),
                                           (96, 64)):
                                nc.sync.dma_start(xr[b0:b0 + 32, :],
                                                  tmp[b1:b1 + 32, :])
                            t1 = QS.tile([128, TPC], BF16, name="t1q",
                                         tag="t1q")
                            nc.vector.tensor_mul(t1[:], tmp[:], cosq_sb[:])
                            nc.vector.tensor_mul(xr[:], xr[:], sinq_sb[:])
                            nc.vector.tensor_add(qr_hk[g][0:64, 2 * lj, :],
                                                 t1[0:64, :], xr[0:64, :])
                            nc.vector.tensor_add(
                                qr_hk[g][64:128, 2 * lj + 1, :],
                                t1[64:128, :], xr[64:128, :])

                # ===================== kv_b (gathered ckv) =====================
                with tc.tile_pool(name="kbps", bufs=2, space="PSUM") as PSB:
                    for h in range(KV):
                        for t in range(4):
                            pk = PSB.tile([128, 512], F32, name="ps_kn",
                                          tag="ps_kn")
                            for k in range(4):
                                nc.tensor.matmul(
                                    pk[:],
                                    wkvb_sb[:, k, 0, h * 128:(h + 1) * 128],
                                    ckvT[k][:, 2 * t:2 * t + 2, :],
                                    start=(k == 0), stop=(k == 3))
                            nc.vector.tensor_copy(
                                knopeT[h][:, 512 * t:512 * (t + 1)], pk[:])
                    for m in range(NB):
                        g, hf = divmod(m, 2)
                        pv = PSB.tile([128, 512], F32, name="ps_v", tag="ps_v")
                        for k in range(4):
                            nc.tensor.matmul(
                                pv[:],
                                ckvT[k][:, g, 128 * hf:128 * hf + 128],
                                wkvb_sb[:, k, 1, :],
                                start=(k == 0), stop=(k == 3))
                        nc.vector.tensor_copy(v_sb[m][:], pv[:])
                ckv_ctx.__exit__(None, None, None)

            # =========================== Attention ==========================
            # Per kv head hk: 4 query heads (2 pairs). Block kb only touches
            # query cols >= 16*kb; the 16-col diagonal band gets the additive
            # mask (identical for every kb). kb<8: pair-wise score matmuls
            # (2*w <= 512); kb>=8: one 4-wide matmul (4*w <= 512). Sums / AV
            # accumulate pair-wise in persistent psum. Normalization closures
            # are deferred past the next hk's first block to keep PE fed.
            pending = []
            pending_b = None
            with tc.tile_pool(name="wop", bufs=1) as WOP:
                # o_proj weights: allocated now (reusing the Q window's
                # freed space), streamed on scalar during attention
                wo_sb = [WOP.tile([128, 2, 4, 512], BF16, name=f"wo{u}")
                         for u in range(8)]
                for u in range(8):
                    nc.scalar.dma_start(wo_sb[u][:], wo_d[u, :, :, :, :])
                att_ctx = tc.tile_pool(name="aps", bufs=1, space="PSUM")
                PSA = att_ctx.__enter__()
                for hk in range(4):
                    ps_av = [PSA.tile([128, 2, TPC], F32, name=f"ps_av{lj}",
                                      tag=f"ps_av{lj}", bufs=2)
                             for lj in range(2)]
                    ps_sum = [PSA.tile([1, 2, TPC], F32, name=f"ps_sum{lj}",
                                       tag=f"ps_sum{lj}") for lj in range(2)]
                    def sums_av(kb, p_t):
                        q0 = 16 * kb
                        for lj in range(2):
                            sl = slice(2 * lj, 2 * lj + 2)
                            nc.tensor.matmul(ps_sum[lj][:, :, q0:], ones_b[:],
                                             p_t[:, sl, q0:], start=(kb == 0),
                                             stop=(kb == NB - 1))
                            nc.tensor.matmul(
                                ps_av[lj][:, :, q0:],
                                v_sb[kb][:, hk * 128:(hk + 1) * 128],
                                p_t[:, sl, q0:], start=(kb == 0),
                                stop=(kb == NB - 1))

                    prev = None
                    for kb in range(NB):
                        q0 = 16 * kb
                        w = TPC - q0
                        kcols = slice(kb * 128, (kb + 1) * 128)
                        p_t = PP.tile([128, 4, TPC], BF16, name="p_t",
                                      tag="p_t")
                        for lj in range(2):
                            sl = slice(2 * lj, 2 * lj + 2)
                            ps_sc = PSA.tile([128, 2, TPC], F32,
                                             name="ps_sc", tag="ps_sc",
                                             bufs=2)
                            nc.tensor.matmul(ps_sc[:, :, q0:],
                                             knopeT[hk][:, kcols],
                                             qn_hk[hk][:, sl, q0:],
                                             start=True, stop=False)
                            nc.tensor.matmul(ps_sc[:, :, q0:],
                                             krot2[:, kcols],
                                             qr_hk[hk][:, sl, q0:],
                                             start=False, stop=True)
                            nc.vector.tensor_add(ps_sc[:, :, q0:q0 + 16],
                                                 ps_sc[:, :, q0:q0 + 16],
                                                 bmask_sb[:, 0:2, :])
                            nc.scalar.activation(p_t[:, sl, q0:],
                                                 ps_sc[:, :, q0:],
                                                 AF.Exp, scale=SCALE)
                        if kb == 0 and pending:
                            # part A of the previous hk's normalization: the
                            # serial DVE reciprocal starts now (before this
                            # hk's first sum matmul recycles the bank)
                            pending_b = pending.pop()()
                        if kb == 4 and pending_b is not None:
                            # part B: by now the reciprocal is done, so the
                            # PE broadcast + evac run without stalling
                            pending_b()
                            pending_b = None
                        if prev is not None:
                            sums_av(*prev)
                        prev = (kb, p_t)
                    sums_av(*prev)

                    def norm_a(hk=hk, ps_av=ps_av, ps_sum=ps_sum):
                        recbs = []
                        for lj in range(2):
                            rec = PP.tile([1, 2, TPC], F32, name="rec_r",
                                          tag="rec_r", bufs=2)
                            nc.vector.reciprocal(rec[:], ps_sum[lj][:])
                            recb = PP.tile([1, 2, TPC], BF16, name="recb_r",
                                           tag="recb_r", bufs=2)
                            nc.vector.tensor_copy(recb[:], rec[:])
                            recbs.append(recb)

                        def norm_b():
                            for lj in range(2):
                                ps_rb = PSA.tile([128, 2, TPC], F32,
                                                 name="ps_rb", tag="ps_sc",
                                                 bufs=2)
                                nc.tensor.matmul(ps_rb[:], ones_row[:],
                                                 recbs[lj][:], start=True,
                                                 stop=True)
                                rb = PP.tile([128, 2, TPC], F32, name="rb",
                                             tag="rb", bufs=2)
                                nc.vector.tensor_copy(rb[:], ps_rb[:])
                                nc.vector.tensor_mul(
                                    qn_hk[hk][:, 2 * lj:2 * lj + 2, :],
                                    ps_av[lj][:], rb[:])
                        return norm_b
                    pending.append(norm_a)
                while pending:
                    pending.pop()()()
                if pending_b is not None:
                    pending_b()
                att_ctx.__exit__(None, None, None)

                # ========================= o_proj =========================
                with tc.tile_pool(name="ops", bufs=2, space="PSUM") as PSB:
                    for n in range(4):
                        pso = [PSB.tile([128, 512], F32, name=f"ps_o{m}",
                                        tag=f"ps_o{m}") for m in range(2)]
                        for h in range(H):
                            u, hh = divmod(h, 2)
                            for m in range(2):
                                nc.tensor.matmul(
                                    pso[m][:],
                                    qn_hk[h // 4][:, h % 4,
                                                  m * 128:(m + 1) * 128],
                                    wo_sb[u][:, hh, n, :], start=(h == 0),
                                    stop=(h == H - 1))
                        for m in range(2):
                            osb = PP.tile([128, 512], F32, name="osb",
                                          tag="osb", bufs=2)
                            nc.vector.tensor_copy(osb[:], pso[m][:])
                            nc.sync.dma_start(out_d[m, n, :, :], osb[:])

    return nc


def kernel(hidden_states, cos, sin, wq_a, q_a_ln_w, wq_b, wkv_a, kv_a_ln_w,
           wkv_b, wo, cache_position, _trace=False):
    global _BUILT, LAST_RESULTS
    hidden_states = np.asarray(hidden_states, dtype=np.float32)
    cos = np.asarray(cos, dtype=np.float32)
    sin = np.asarray(sin, dtype=np.float32)
    wq_a = np.asarray(wq_a, dtype=np.float32)
    q_a_ln_w = np.asarray(q_a_ln_w, dtype=np.float32)
    wq_b = np.asarray(wq_b, dtype=np.float32)
    wkv_a = np.asarray(wkv_a, dtype=np.float32)
    kv_a_ln_w = np.asarray(kv_a_ln_w, dtype=np.float32)
    wkv_b = np.asarray(wkv_b, dtype=np.float32)
    wo = np.asarray(wo, dtype=np.float32)
    cp = np.asarray(cache_position).astype(np.int64)

    def b16(x):
        return np.ascontiguousarray(np.asarray(x, np.float32).astype(bfloat16))

    # ---- host-side prep (layout/sharding only) ----
    h = hidden_states[0]                       # [S, D]
    cos_sel = cos[0][cp]                       # [S, DR]
    sin_sel = sin[0][cp]
    sgn = np.concatenate([-np.ones(DR // 2), np.ones(DR // 2)]
                         ).astype(np.float32)[None, :]
    sinS = sin_sel * sgn                       # rotate_half sign folded

    # wqa granules: [q, half, p, kk, mc] = wq_a[128*(4q+kk)+p, 768*half+mc]
    wqa_dev = b16(wq_a.reshape(4, 4, 128, 2, 768).transpose(0, 3, 2, 1, 4))
    # wqb tiles (ln folded): per g: nope heads 4g..4g+3 then rope pairs
    wqb_eff = wq_b * q_a_ln_w[:, None]
    wqb_r3 = wqb_eff.reshape(QR, H, DQK)
    qb_g = np.concatenate(
        [wqb_r3[:, :, :DN].reshape(QR, 4, 512),
         wqb_r3[:, :, DN:].reshape(QR, 4, 256)], axis=2)  # [QR, 4, 768]
    wqb_dev = b16(qb_g.reshape(2, 6, 128, 4, 768)
                  .transpose(3, 0, 2, 1, 4))              # [g, hf, p, kc, u]
    # wkva tiles: [q, p, kk, m]
    wkva_dev = b16(wkv_a.reshape(4, 4, 128, KVR + DR).transpose(0, 2, 1, 3))
    # wkvb tile: [p, k, {k-nope|v}, m]
    wkvb_eff = wkv_b * kv_a_ln_w[:, None]      # [KVR, KV*(DN+DV)]
    wkvb_r = wkvb_eff.reshape(KVR, KV, DN + DV)
    wkvbk = wkvb_r[:, :, :DN].reshape(4, 128, KV * DN)
    wkvbv = wkvb_r[:, :, DN:].reshape(4, 128, KV * DV)
    wkvb_dev = b16(np.stack([wkvbk, wkvbv], axis=2).transpose(1, 0, 2, 3))
    # wo tiles: [u, p, hh, n, m]
    wo_dev = b16(wo.reshape(8, 2, 128, 4, 512).transpose(0, 2, 1, 3, 4))

    hT = h.T                                   # [D, S]
    in_maps = []
    for c in range(NC_):
        qt = np.arange(c, S, NC_)              # this core's query tokens
        kt = np.arange(256 * c, 256 * (c + 1))  # this core's kv tokens
        hq_dev = b16(hT[:, qt].reshape(16, 128, TPC).transpose(1, 0, 2))
        hkv_dev = b16(hT[:, kt].reshape(16, 128, TPC).transpose(1, 0, 2))
        cossin_kv = b16(np.stack(
            [cos_sel[kt].T, sinS[kt].T], axis=1))        # [64, 2, 256]
        cq = cos_sel[qt].T
        sq = sinS[qt].T
        cosq_dev = b16(np.concatenate([cq, cq], axis=0))
        sinq_dev = b16(np.concatenate([sq, sq], axis=0))
        k_ = np.arange(128)[:, None]
        i_ = np.arange(16)[None, :]
        bm = np.where(k_ <= 8 * i_ + c, 0.0, NEG).astype(np.float32)
        bm_dev = np.ascontiguousarray(
            np.repeat(bm[:, None, :], 4, axis=1))        # [128, 4, 16]
        in_maps.append({
            "hq": hq_dev, "hkv": hkv_dev, "wqa": wqa_dev, "wqb": wqb_dev,
            "wkva": wkva_dev, "wkvb": wkvb_dev, "wo": wo_dev,
            "cossin_kv": cossin_kv, "cosq": cosq_dev, "sinq": sinq_dev,
            "bmask": bm_dev,
        })

    if _BUILT is None:
        _BUILT = _build()
    nc = _BUILT

    res = run_bass_kernel_spmd(nc, in_maps, core_ids=list(range(NC_)),
                               trace=_trace)
    LAST_RESULTS = res

    out_full = np.empty((S, D), dtype=np.float32)
    for c in range(NC_):
        o = res.results[c]["out"]              # [2, 4, 128, 512]
        o = o.transpose(0, 2, 1, 3).reshape(TPC, D)
        out_full[c::NC_] = o                   # row m <-> token 8m+c
    return out_full[None]                      # [1, S, D]
